# revision 13
# baseline (speedup 1.0000x reference)
"""AffinityLoss (torchdistill) Trainium2 kernel.

loss = mean_b [ sum_c sqrt(2 - 2*cos^2(s_bc, t_bc)) / HW ]

with s_bc, t_bc the HW-dim spatial vectors of channel c of sample b.
cos(s, t) = <s,t> / (||s|| ||t||), so per (b, c) we only need the three
dot products ss, tt, st over the 16384-element spatial dim.

Sharding: data-parallel over the batch dim B=8 -> one sample per
NeuronCore (8 cores). Per core, channels live on SBUF partitions
(2 chunks of 128) and the spatial dim is tiled along the free dim.

Per spatial tile [128, F]:
  - ScalarE (ACT): Square activation with accum_out -> partial ss, tt
  - VectorE (DVE): tensor_tensor_reduce(mult, add)  -> partial st
Both write their full-size `out` through a stride-0 broadcast dummy, so
no scratch SBUF and no extra write bandwidth is spent. The three
reductions stream at engine line rate and hide entirely under the HBM
DMA (~32 MiB/core), which is the roofline for this memory-bound loss.

Epilogue (per core, tiny [128, 2] math): cos^2 = st^2/(ss*tt),
v = max(2 - 2 cos^2, 0), w = sqrt(v), then a ones-vector matmul on the
TensorEngine reduces over the 128 partitions -> [1, 2] per core.
Host: sum the 16 numbers, divide by HW*B.
"""

import numpy as np

import concourse.bacc as bacc
import concourse.tile as tile
from concourse import mybir
from concourse.bass_utils import run_bass_kernel_spmd

B, C, H, W = 8, 256, 128, 128
HW = H * W           # 16384 spatial elements per channel
P = 128              # SBUF partitions
NCORES = 8

F = 8192             # spatial tile width (per-DMA: 128 x 8192 x 4B = 4 MiB)


def _tile_schedule(C, HW, F):
    """(cc, offset, width, col) list: wide tiles for DMA efficiency, with the
    final tiles narrowed so the post-last-DMA compute tail is short."""
    CC = C // P
    sched = []
    col = 0
    col_ranges = []
    for cc in range(CC):
        if HW % F == 0 and HW // F >= 1:
            widths = [F] * (HW // F)
            if cc == CC - 1 and F >= 8192:
                # split the last wide tile so the post-last-DMA compute
                # tail shrinks: F -> F/2, F/4, F/8, F/8
                widths = widths[:-1] + [F // 2, F // 4, F // 8, F // 8]
        else:
            widths = [F] * (HW // F) + ([HW % F] if HW % F else [])
        c0 = col
        off = 0
        for w in widths:
            sched.append((cc, off, w, col))
            off += w
            col += 1
        assert off == HW
        col_ranges.append((c0, col))
    return sched, col_ranges


def build_program(C=C, HW=HW, F=F, ncores=NCORES):
    f32 = mybir.dt.float32
    CC = C // P          # channel chunks (channels on partitions)

    nc = bacc.Bacc("TRN2", target_bir_lowering=False, debug=False,
                   num_devices=ncores)
    s_d = nc.dram_tensor("student", [C, HW], f32, kind="ExternalInput").ap()
    t_d = nc.dram_tensor("teacher", [C, HW], f32, kind="ExternalInput").ap()
    out_d = nc.dram_tensor("out", [1, CC], f32, kind="ExternalOutput").ap()

    sched, col_ranges = _tile_schedule(C, HW, F)
    NCOL = len(sched)

    with tile.TileContext(nc) as tc:
        with (
            tc.tile_pool(name="io", bufs=2) as io,
            tc.tile_pool(name="small", bufs=1) as small,
            tc.tile_pool(name="psum", bufs=1, space="PSUM") as psum,
        ):
            ss_acc = small.tile([P, NCOL], f32)
            tt_acc = small.tile([P, NCOL], f32)
            st_acc = small.tile([P, NCOL], f32)
            dummy_act = small.tile([P, 1], f32)
            dummy_dve = small.tile([P, 1], f32)
            ones = small.tile([P, 1], f32)
            nc.vector.memset(ones, 1.0)

            # Touch Sqrt first so the ACT table set loads while the first
            # DMAs stream (Square rides in the same set as filler).
            nc.vector.memset(dummy_act, 1.0)
            nc.scalar.sqrt(dummy_act, dummy_act)

            ss = small.tile([P, CC], f32)
            tt = small.tile([P, CC], f32)
            st = small.tile([P, CC], f32)

            for cc, off, w, col in sched:
                s_tile = io.tile([P, F], f32, tag="s")
                nc.sync.dma_start(
                    out=s_tile[:, :w],
                    in_=s_d[cc * P:(cc + 1) * P, off:off + w],
                )
                t_tile = io.tile([P, F], f32, tag="t")
                # second DGE ring (SWDGE) so transfer ramps overlap the
                # sync-ring stream instead of convoying behind it
                nc.gpsimd.dma_start(
                    out=t_tile[:, :w],
                    in_=t_d[cc * P:(cc + 1) * P, off:off + w],
                )

                nc.scalar.activation(
                    out=dummy_act.broadcast_to(s_tile[:, :w].shape),
                    in_=s_tile[:, :w],
                    func=mybir.ActivationFunctionType.Square,
                    accum_out=ss_acc[:, col:col + 1],
                )
                nc.scalar.activation(
                    out=dummy_act.broadcast_to(t_tile[:, :w].shape),
                    in_=t_tile[:, :w],
                    func=mybir.ActivationFunctionType.Square,
                    accum_out=tt_acc[:, col:col + 1],
                )
                # NOTE: tensor_tensor_reduce wedges the exec unit on this
                # runtime build; scalar_tensor_tensor + accum_out is the
                # same single-pass fused multiply-reduce on the DVE.
                nc.vector.scalar_tensor_tensor(
                    out=dummy_dve.broadcast_to(s_tile[:, :w].shape),
                    in0=s_tile[:, :w],
                    scalar=1.0,
                    in1=t_tile[:, :w],
                    op0=mybir.AluOpType.mult,
                    op1=mybir.AluOpType.mult,
                    accum_out=st_acc[:, col:col + 1],
                )

                # fold this cc's partials as soon as its last tile is in —
                # all but the last cc's reduces hide under the next stream
                cc_end = col_ranges[cc][1] - 1
                if col == cc_end:
                    c0, c1 = col_ranges[cc]
                    for acc_t, red_t in ((ss_acc, ss), (tt_acc, tt),
                                         (st_acc, st)):
                        nc.vector.tensor_reduce(
                            out=red_t[:, cc:cc + 1],
                            in_=acc_t[:, c0:c1],
                            axis=mybir.AxisListType.X,
                            op=mybir.AluOpType.add,
                        )

            # ---- epilogue: [128, CC] closed-form per channel ----

            denom = small.tile([P, CC], f32)
            nc.vector.tensor_mul(denom, ss, tt)          # ss*tt
            nc.vector.reciprocal(denom, denom)           # 1/(ss*tt)
            cos2 = small.tile([P, CC], f32)
            nc.vector.tensor_mul(cos2, st, st)           # st^2
            nc.vector.tensor_mul(cos2, cos2, denom)      # cos^2
            v = small.tile([P, CC], f32)
            nc.vector.tensor_scalar(                     # 2 - 2*cos^2
                v, cos2, -2.0, 2.0,
                op0=mybir.AluOpType.mult, op1=mybir.AluOpType.add,
            )
            nc.vector.tensor_scalar_max(v, v, 0.0)
            nc.scalar.sqrt(v, v)

            ps = psum.tile([1, CC], f32)
            nc.tensor.matmul(ps, lhsT=ones, rhs=v, start=True, stop=True)
            res = small.tile([1, CC], f32)
            nc.vector.tensor_copy(res, ps)
            nc.sync.dma_start(out=out_d, in_=res)

    nc.finalize()
    return nc


_PROGRAM = None


def _get_program():
    global _PROGRAM
    if _PROGRAM is None:
        _PROGRAM = build_program()
    return _PROGRAM


def kernel(student: np.ndarray, teacher: np.ndarray) -> np.ndarray:
    s = np.ascontiguousarray(np.asarray(student, dtype=np.float32)).reshape(B, C, HW)
    t = np.ascontiguousarray(np.asarray(teacher, dtype=np.float32)).reshape(B, C, HW)

    nc = _get_program()
    in_maps = [{"student": s[i], "teacher": t[i]} for i in range(NCORES)]
    results = run_bass_kernel_spmd(nc, in_maps, list(range(NCORES))).results

    total = sum(float(results[i]["out"].sum()) for i in range(NCORES))
    return np.float32(total / (HW * B))
